# revision 1
# baseline (speedup 1.0000x reference)
"""Multi-head self-attention (8 heads, head_dim 64, n=4096, dim=256) on 8
Trainium2 NeuronCores.

Sharding: one attention head per core (tensor parallel on the heads axis of
to_qkv / to_out). Each core:
  A) computes qk = [q; k] (128 x 4096 bf16, q scaled by 184.665/8 so logits
     arrive pre-scaled for both exp paths), derives the dual layout kq by
     SBUF->SBUF partition-swap DMA, and v^T (4096 x 64 bf16, augmented with a
     ones column so the softmax denominator falls out of the PE matmul),
  B) streams the 4096x4096 attention for its head: sim = k^T q in [j, i]
     layout with paired row-group matmuls (tile_position 0/64) so two j-tiles
     stream concurrently.  exp is SPLIT across two engines: ScalarE computes
     exact exp (activation scale un-does the 184.665 pre-scale) while VectorE
     computes Schraudolph fast-exp (add 16250.2, round-convert to int16,
     bitcast to bf16 = 2^(l*log2e) within ~3%).  j-tiles stream in units
     of 4 (two co-issued sim pairs into 2-slot PSUM tiles, exp split 16/16
     between the engines, out' matmuls batched by 4) so PE/ACT/DVE all stay
     busy; the attention-weighted value sum accumulates over j in PSUM.
  C) normalizes by the fused row sum read directly from PSUM: the sums row
     is DMA-respread over 128 lanes for a wide reciprocal, then broadcast
     back (the last block instead uses a latency-lean in-SBUF path: DVE
     32x32 transpose spread + reciprocal + K=1 matmul replicate, since the
     AllToAll waits on it).  The AllToAll then gives every core the full
     512-channel hidden state for its own 512-token slice, followed by the
     output projection and bias.
The host only reshapes/slices/casts inputs per core and concatenates the 8
disjoint token slices of the output.
"""

import os
import sys
from contextlib import ExitStack

for _p in ("/opt/trn_rl_repo",):
    if os.path.isdir(_p) and _p not in sys.path:
        sys.path.append(_p)

import ml_dtypes
import numpy as np

import concourse.bass as bass
import concourse.mybir as mybir
import concourse.tile as tile
from concourse import bacc
from concourse.bass_utils import run_bass_kernel_spmd

HEADS = 8
HD = 64           # head dim
DIM = 256         # model dim
N = 4096          # tokens (64*64)
HID = HEADS * HD  # 512
NB = 8            # token blocks
BLK = N // NB     # 512
NJ = N // 128     # 32 j-tiles of 128
N_CORES = 8

F32 = mybir.dt.float32
BF16 = mybir.dt.bfloat16
I16 = mybir.dt.int16
EXP = mybir.ActivationFunctionType.Exp

SCH_SCALE = 184.6650292   # 128 / ln(2): absorbed into wq on the host
SCH_BIAS = 16250.2        # 127*128 - 5.8 (Schraudolph magic, bf16 mantissa)
ACT_SCALE = 1.0 / SCH_SCALE


def build_program():
    nc = bacc.Bacc("TRN2", target_bir_lowering=False, debug=False,
                   num_devices=N_CORES)
    x_d = nc.declare_dram_parameter("x", [DIM, N], BF16, isOutput=False)
    # columns: [wq|wk | unused | wv] (wq pre-scaled by SCH_SCALE/8)
    wqkvT_d = nc.declare_dram_parameter("wqkvT", [DIM, 320], BF16,
                                        isOutput=False)
    woT_d = nc.declare_dram_parameter("woT", [HID, DIM], BF16, isOutput=False)
    b_d = nc.declare_dram_parameter("bout", [DIM], F32, isOutput=False)
    y_d = nc.declare_dram_parameter("y", [DIM, BLK], F32, isOutput=True)

    with tile.TileContext(nc) as tc, ExitStack() as ctx:
        const = ctx.enter_context(tc.tile_pool(name="const", bufs=1))
        sbA = ctx.enter_context(tc.tile_pool(name="sbA", bufs=1))
        peA_p = ctx.enter_context(tc.tile_pool(name="peA", bufs=5))
        peD_p = ctx.enter_context(tc.tile_pool(name="peD", bufs=5))
        psml = ctx.enter_context(tc.tile_pool(name="psml", bufs=3))
        dram = ctx.enter_context(tc.tile_pool(name="dram", bufs=1,
                                              space="DRAM"))
        psO = ctx.enter_context(tc.tile_pool(name="psO", bufs=2,
                                             space="PSUM"))

        # ---- constants / persistent SBUF ----
        wqkvT_sb = const.tile([128, 2, 320], BF16)
        nc.sync.dma_start(wqkvT_sb[:],
                          wqkvT_d.rearrange("(c p) m -> p c m", p=128))
        woT_sb = const.tile([128, 4, DIM], BF16)
        nc.sync.dma_start(woT_sb[:],
                          woT_d.rearrange("(c p) m -> p c m", p=128))
        b_sb = const.tile([128, 2], F32)
        nc.sync.dma_start(b_sb[:], b_d.rearrange("(m p) -> p m", p=128))

        ones_sb = const.tile([128, HD], BF16)
        nc.vector.memset(ones_sb[:], 1.0)
        x_sb = sbA.tile([128, 2, N], BF16)
        qk_sb = sbA.tile([128, N], BF16)   # partitions 0:64 = q, 64:128 = k
        kq_sb = sbA.tile([128, N], BF16)   # partitions 0:64 = k, 64:128 = q
        # v^T augmented with a ones column: [j, 0:64] = v^T, [j, 64] = 1
        vaug_sb = sbA.tile([128, NJ, 66], BF16)
        nc.vector.memset(vaug_sb[:, :, 64:65], 1.0)

        a2a_in = dram.tile([NB, HD, BLK], BF16)
        a2a_out = dram.tile([NB, HD, BLK], BF16)
        warm_in = dram.tile([128, 4], F32)
        warm_out = dram.tile([128, 4], F32)
        # tiny warm-up collective: absorbs CC init cost under the preamble
        # (an extra warm-up AllToAll measurably SLOWS the real one - don't)
        nc.gpsimd.collective_compute(
            "AllReduce", mybir.AluOpType.add,
            replica_groups=[list(range(N_CORES))],
            ins=[warm_in.opt()], outs=[warm_out.opt()])

        pending = []  # [countdown, fn] emitted in order once countdown <= 0

        def schedule(fn, delay):
            pending.append([delay, fn])

        def tick():
            for it in pending:
                it[0] -= 1
            for it in [it for it in pending if it[0] <= 0]:
                pending.remove(it)
                it[1]()

        def drain():
            while pending:
                pending.pop(0)[1]()

        def emit_stage_a(b, pool):
            bs = slice(b * BLK, (b + 1) * BLK)
            # x loads run one block ahead (block b+1 issued here; block 0
            # hoisted before the loop) so the projections never wait on DMA
            if b + 1 < NB:
                nbs = slice((b + 1) * BLK, (b + 2) * BLK)
                for c in range(2):
                    nc.sync.dma_start(x_sb[:, c, nbs],
                                      x_d[c * 128:(c + 1) * 128, nbs])
            # qk in slot 0 of a shared pair tile; the 4 v accumulators fit
            # in the first half of slot 1 (1 KB of the 2 KB bank)
            ps = pool.tile([128, 2, BLK], F32, tag="psg", name=f"psa_{b}")
            ps_qk = ps[:, 0, :]
            for c in range(2):
                nc.tensor.matmul(ps_qk, wqkvT_sb[:, c, 0:128],
                                 x_sb[:, c, bs],
                                 start=(c == 0), stop=(c == 1))
            nc.vector.tensor_copy(qk_sb[:, bs], ps_qk)
            # dual layout by partition-swap DMA instead of 2 extra matmuls
            # (split across sync/scalar queues; gpsimd would stall them
            # behind the warm-up collective's rendezvous)
            nc.sync.dma_start(kq_sb[0:64, bs], qk_sb[64:128, bs])
            nc.scalar.dma_start(kq_sb[64:128, bs], qk_sb[0:64, bs])
            ps_v = ps[:, 1, 0:4 * HD].rearrange("p (t d) -> p t d", t=4)
            for t in range(4):
                nt = b * 4 + t
                for c in range(2):
                    nc.tensor.matmul(
                        ps_v[:, t, :],
                        x_sb[:, c, nt * 128:(nt + 1) * 128],
                        wqkvT_sb[:, c, 256:320],
                        start=(c == 0), stop=(c == 1))
            nc.vector.tensor_copy(vaug_sb[:, 4 * b:4 * b + 4, 0:64], ps_v[:])

        def emit_sim(i, j, psg, t):
            # sim matmul, row-packed by parity: even j on array rows 0:64,
            # odd j on rows 64:128 -> each (even, odd) pair streams
            # concurrently through the PE.
            isl = slice(i * BLK, (i + 1) * BLK)
            if j % 2 == 0:
                nc.tensor.matmul(psg[:, t, :],
                                 kq_sb[0:64, j * 128:(j + 1) * 128],
                                 qk_sb[0:64, isl],
                                 start=True, stop=True,
                                 tile_position=(0, 0))
            else:
                nc.tensor.matmul(psg[:, t, :],
                                 qk_sb[64:128, j * 128:(j + 1) * 128],
                                 kq_sb[64:128, isl],
                                 start=True, stop=True,
                                 tile_position=(64, 0))

        def emit_exp_outs(i, js, nA, psg, ps_out, delay):
            # exp on ScalarE for the first nA tiles (activation scale
            # un-does the host pre-scale), Schraudolph fast-exp on VectorE
            # for the rest, then the out' accumulation matmuls (delayed).
            gsz = len(js)
            nD = gsz - nA
            pea = ped = None
            if nD > 0:
                ped = peD_p.tile([128, 3, BLK], I16, tag="ped",
                                 name=f"ped_{i}_{js[0]}")
                nc.vector.tensor_scalar_add(ped[:, 0:nD, :],
                                            psg[:, nA:gsz, :], SCH_BIAS)
            if nA > 0:
                pea = peA_p.tile([128, 3, BLK], BF16, tag="pea",
                                 name=f"pea_{i}_{js[0]}")
                nc.scalar.activation(pea[:, 0:nA, :], psg[:, 0:nA, :], EXP,
                                     scale=ACT_SCALE)

            def mk_outp():
                for t2, j2 in enumerate(js):
                    if t2 < nA:
                        rhs = pea[:, t2, :]
                    else:
                        rhs = ped[:, t2 - nA, :].bitcast(BF16)
                    nc.tensor.matmul(ps_out[0:65, :],
                                     vaug_sb[:, j2, 0:65],
                                     rhs,
                                     start=(j2 == 0), stop=(j2 == NJ - 1))
            schedule(mk_outp, delay)

        def emit_unit(i, u, ps_out, pool):
            # one unit of 4 j-tiles: two (even, odd) sim pairs into two
            # 2-slot psg tiles (a pair shares a tile, so the scheduler
            # cannot split it), exp for pair A on ScalarE and pair B on
            # VectorE, and the four out' matmuls batched to halve
            # LDWEIGHTS row-conflict stalls at the sim/out boundaries.
            j0 = 4 * u
            psgA = pool.tile([128, 2, BLK], F32, tag="psg",
                             name=f"psgA_{i}_{u}")
            emit_sim(i, j0, psgA, 0)
            emit_sim(i, j0 + 1, psgA, 1)
            psgB = pool.tile([128, 2, BLK], F32, tag="psg",
                             name=f"psgB_{i}_{u}")
            emit_sim(i, j0 + 2, psgB, 0)
            emit_sim(i, j0 + 3, psgB, 1)
            emit_exp_outs(i, [j0, j0 + 1], 2, psgA, ps_out, 2)
            emit_exp_outs(i, [j0 + 2, j0 + 3], 0, psgB, ps_out, 2)
            tick()

        def emit_iblock(i, ps_out, pool):
            for u in range(NJ // 4):
                emit_unit(i, u, ps_out, pool)

        def emit_norm(i, ps_out):
            # row sums live on partition 64 of ps_out; read PSUM directly
            # (psO is double-buffered so the bank frees at leisure).
            s_sb = psml.tile([128, BLK], F32, tag="ssb", name=f"ssb_{i}")

            if i == NB - 1:
                # tail block: the a2a is waiting on this chain, so avoid
                # DRAM bounces entirely.  Spread the sums row over 32
                # lanes with the DVE 32x32 transpose, reciprocal there,
                # transpose back, then replicate with a K=1 bf16 matmul.
                st_sb = psml.tile([128, BLK], F32, tag="stt", name="st_t")
                rt_sb = psml.tile([128, BLK], F32, tag="rtt", name="rt_t")
                rb_sb = psml.tile([128, BLK], F32, tag="rbb", name="rb_t")
                rh_sb = psml.tile([128, BLK], BF16, tag="rhh", name="rh_t")
                oall = psml.tile([HD, BLK], F32, tag="rrep", name="oall_t")

                def mk_tail():
                    nc.vector.tensor_copy(s_sb[64:65, :], ps_out[64:65, :])
                    nc.scalar.copy(oall[:], ps_out[0:HD, :])
                    nc.vector.transpose(st_sb[64:96, :], s_sb[64:96, :])
                    stv = st_sb[64:96, :].rearrange("p (a b) -> p a b", b=32)
                    rtv = rt_sb[64:96, :].rearrange("p (a b) -> p a b", b=32)
                    nc.vector.reciprocal(rtv[:, :, 0:1], stv[:, :, 0:1])
                    nc.vector.transpose(rb_sb[64:96, :], rt_sb[64:96, :])
                    nc.vector.tensor_copy(rh_sb[64:65, :], rb_sb[64:65, :])
                    ps_r = psO.tile([128, BLK], F32, tag="psout",
                                    name="psr_tail")
                    nc.tensor.matmul(ps_r[0:HD, :], ones_sb[64:65, 0:HD],
                                     rh_sb[64:65, :], start=True, stop=True)
                    outn = psml.tile([HD, BLK], BF16, tag="outn",
                                     name="outn_t")
                    nc.vector.tensor_mul(outn[:], oall[:], ps_r[0:HD, :])
                    nc.sync.dma_start(a2a_in[i], outn[:])
                schedule(mk_tail, 2)
                return

            s4_sb = psml.tile([128, 4], F32, tag="s4", name=f"s4_{i}")
            r4_sb = psml.tile([128, 4], F32, tag="r4", name=f"r4_{i}")
            rrow = dram.tile([BLK], F32, tag="rrow", bufs=2,
                             name=f"rrow_{i}")
            rrec = dram.tile([BLK], F32, tag="rrec", bufs=2,
                             name=f"rrec_{i}")

            def mk_norm_a():
                # sums row -> DRAM -> respread over 128 lanes so the
                # reciprocal runs wide instead of on one partition
                nc.vector.tensor_copy(s_sb[64:65, :], ps_out[64:65, :])
                nc.sync.dma_start(rrow[:], s_sb[64:65, :])
            schedule(mk_norm_a, 2)

            def mk_norm_a2():
                nc.sync.dma_start(s4_sb[:],
                                  rrow.rearrange("(p c) -> p c", p=128))
                nc.vector.reciprocal(r4_sb[:], s4_sb[:])
                nc.sync.dma_start(rrec[:], r4_sb[:])
            schedule(mk_norm_a2, 4)

            def mk_norm_b():
                outn = psml.tile([HD, BLK], BF16, tag="outn",
                                 name=f"outn_{i}")
                rrep_sb = psml.tile([HD, BLK], F32, tag="rrep",
                                    name=f"rrep_{i}")
                nc.sync.dma_start(
                    rrep_sb[:],
                    rrec.rearrange("(o n) -> o n", o=1).broadcast_to(
                        (HD, BLK)))
                nc.vector.tensor_mul(outn[:], ps_out[0:HD, :],
                                     rrep_sb[:])
                nc.sync.dma_start(a2a_in[i], outn[:])
            schedule(mk_norm_b, 6)

        # ---- stage A interleaved with i-block 0: unit u of i0 needs
        # only stage-A block u's outputs, so the i0 units slot in one
        # block behind the projections (all in the shared psB3 pool) -----
        with tc.tile_pool(name="psB3", bufs=3, space="PSUM") as psB3:
            # warm the PE clock-gate during the preamble DMA wait so the
            # projections run at 2.4 GHz from block 0
            ps_w = psB3.tile([128, 2, BLK], F32, tag="psg", name="ps_warmup")
            for w in range(12):
                nc.tensor.matmul(ps_w[:, 0, 0:320], wqkvT_sb[:, 0, 0:128],
                                 wqkvT_sb[:, 1, :], start=True, stop=True)
            ps_out0 = psO.tile([128, BLK], F32, tag="psout", name="psout_0")
            for c in range(2):
                nc.sync.dma_start(x_sb[:, c, 0:BLK],
                                  x_d[c * 128:(c + 1) * 128, 0:BLK])
            u0 = 0
            for b in range(NB):
                emit_stage_a(b, psB3)
                if b >= 1:
                    emit_unit(0, u0, ps_out0, psB3)
                    u0 += 1
            while u0 < NJ // 4:
                emit_unit(0, u0, ps_out0, psB3)
                u0 += 1
            emit_norm(0, ps_out0)

            # ---- i-blocks 1..7 ------------------------------------------
            for i in range(1, NB):
                ps_out = psO.tile([128, BLK], F32, tag="psout",
                                  name=f"psout_{i}")
                emit_iblock(i, ps_out, psB3)
                emit_norm(i, ps_out)
            drain()

            # ---- stage C: AllToAll over token blocks + output projection --
            nc.gpsimd.collective_compute(
                "AllToAll", mybir.AluOpType.bypass,
                replica_groups=[list(range(N_CORES))],
                ins=[a2a_in.opt()], outs=[a2a_out.opt()])

            rhs_sb = sbA.tile([128, 4, BLK], BF16)
            a2a_r = a2a_out.rearrange("(c a) d t -> (a d) c t", c=4, a=2)
            ps_yt = psB3.tile([128, 2, BLK], F32, tag="psg", name="psy")
            for c in range(4):
                nc.gpsimd.dma_start(rhs_sb[:, c, :], a2a_r[:, c, :])
                for m in range(2):
                    nc.tensor.matmul(ps_yt[:, m, :],
                                     woT_sb[:, c, m * 128:(m + 1) * 128],
                                     rhs_sb[:, c, :],
                                     start=(c == 0), stop=(c == 3))
            for m in range(2):
                y_sb = psml.tile([128, BLK], F32, tag="ysb", name=f"ysb_{m}")
                nc.vector.tensor_scalar_add(y_sb[:], ps_yt[:, m, :],
                                            b_sb[:, m:m + 1])
                nc.sync.dma_start(y_d[m * 128:(m + 1) * 128, :], y_sb[:])

    nc.compile()
    return nc


def _make_in_maps(x, w_qkv, w_out, b_out):
    x2 = np.ascontiguousarray(
        np.asarray(x, np.float32).reshape(DIM, N)).astype(ml_dtypes.bfloat16)
    w_qkv = np.asarray(w_qkv, np.float32)
    scale = (HD ** -0.5) * SCH_SCALE
    woT = np.ascontiguousarray(np.asarray(w_out, np.float32).T).astype(
        ml_dtypes.bfloat16)
    b = np.ascontiguousarray(np.asarray(b_out, np.float32).reshape(DIM))
    in_maps = []
    for h in range(N_CORES):
        wq = w_qkv[h * HD:(h + 1) * HD] * scale
        wk = w_qkv[HID + h * HD:HID + (h + 1) * HD]
        wv = w_qkv[2 * HID + h * HD:2 * HID + (h + 1) * HD]
        wqkvT = np.ascontiguousarray(
            np.concatenate([wq.T, wk.T, wk.T, wq.T, wv.T], axis=1),
            np.float32).astype(ml_dtypes.bfloat16)
        in_maps.append({"x": x2, "wqkvT": wqkvT, "woT": woT, "bout": b})
    return in_maps


def _assemble(results):
    y = np.concatenate([results[h]["y"] for h in range(N_CORES)], axis=1)
    return np.ascontiguousarray(y.reshape(1, DIM, 64, 64).astype(np.float32))


def kernel(x, w_qkv, w_out, b_out):
    nc = build_program()
    in_maps = _make_in_maps(x, w_qkv, w_out, b_out)
    res = run_bass_kernel_spmd(nc, in_maps, list(range(N_CORES)))
    return _assemble(res.results)


def run_traced(x, w_qkv, w_out, b_out, trace_cores=None):
    """Test-harness entry: also returns BassKernelResults with exec_time_ns."""
    nc = build_program()
    in_maps = _make_in_maps(x, w_qkv, w_out, b_out)
    res = run_bass_kernel_spmd(nc, in_maps, list(range(N_CORES)), trace=True,
                               trace_cores=trace_cores)
    return _assemble(res.results), res



# revision 22
# speedup vs baseline: 1.0782x; 1.0782x over previous
"""Multi-head self-attention (8 heads, head_dim 64, n=4096, dim=256) on 8
Trainium2 NeuronCores.

Sharding: one attention head per core (tensor parallel on the heads axis of
to_qkv / to_out). Each core:
  A) computes qk = [q; k] (128 x 4096 bf16, q scaled by 184.665/8 so logits
     arrive pre-scaled for both exp paths), derives the dual layout kq by
     SBUF->SBUF partition-swap DMA, and v^T (4096 x 64 bf16, augmented with a
     ones column so the softmax denominator falls out of the PE matmul),
  B) streams the 4096x4096 attention for its head: sim = k^T q in [j, i]
     layout with paired row-group matmuls (tile_position 0/64) so two j-tiles
     stream concurrently.  exp is SPLIT across two engines: ScalarE computes
     exact exp (activation scale un-does the 184.665 pre-scale) while VectorE
     computes Schraudolph fast-exp (add 16250.2, round-convert to int16,
     bitcast to bf16 = 2^(l*log2e) within ~3%).  j-tiles stream in units
     of 4 (two co-issued sim pairs into 2-slot PSUM tiles, exp split 16/16
     between the engines, out' matmuls batched by 4) so PE/ACT/DVE all stay
     busy; the attention-weighted value sum accumulates over j in PSUM.
  C) normalizes by the fused row sum read directly from PSUM: the sums row
     is DMA-respread over 128 lanes for a wide reciprocal, then broadcast
     back (the last block instead uses a latency-lean in-SBUF path: DVE
     32x32 transpose spread + reciprocal + K=1 matmul replicate, since the
     AllToAll waits on it).  The AllToAll then gives every core the full
     512-channel hidden state for its own 512-token slice, followed by the
     output projection and bias.
The host only reshapes/slices/casts inputs per core and concatenates the 8
disjoint token slices of the output.
"""

import os
import sys
from contextlib import ExitStack

for _p in ("/opt/trn_rl_repo",):
    if os.path.isdir(_p) and _p not in sys.path:
        sys.path.append(_p)

import ml_dtypes
import numpy as np

import concourse.bass as bass
import concourse.mybir as mybir
import concourse.tile as tile
from concourse import bacc
from concourse.bass_utils import run_bass_kernel_spmd

HEADS = 8
HD = 64           # head dim
DIM = 256         # model dim
N = 4096          # tokens (64*64)
HID = HEADS * HD  # 512
NB = 8            # token blocks
BLK = N // NB     # 512
NJ = N // 128     # 32 j-tiles of 128
N_CORES = 8

F32 = mybir.dt.float32
BF16 = mybir.dt.bfloat16
I16 = mybir.dt.int16
FP8 = mybir.dt.float8e4
EXP = mybir.ActivationFunctionType.Exp

SCH_SCALE = 184.6650292   # 128 / ln(2): absorbed into wq on the host
# all exp outputs are scaled by 2^-OUT_SHIFT so the exact-exp path fits
# fp8e4m3 (max ~240; raw exp peaks ~e^6).  The softmax denominator picks
# up the same factor through the ones column, so it cancels in the
# normalization.
OUT_SHIFT = 5
SCH_BIAS = 16250.2 - 128 * OUT_SHIFT  # 127*128 - 5.8 (Schraudolph magic)
ACT_SCALE = 1.0 / SCH_SCALE
EXP_BIAS = -OUT_SHIFT * 0.6931471805599453


def build_program():
    nc = bacc.Bacc("TRN2", target_bir_lowering=False, debug=False,
                   num_devices=N_CORES)
    x_d = nc.declare_dram_parameter("x", [DIM, N], BF16, isOutput=False)
    # columns: [wq|wk | unused | wv] (wq pre-scaled by SCH_SCALE/8)
    wqkvT_d = nc.declare_dram_parameter("wqkvT", [DIM, 320], BF16,
                                        isOutput=False)
    woT_d = nc.declare_dram_parameter("woT", [HID, DIM], BF16, isOutput=False)
    b_d = nc.declare_dram_parameter("bout", [DIM], F32, isOutput=False)
    y_d = nc.declare_dram_parameter("y", [DIM, BLK], F32, isOutput=True)

    with tile.TileContext(nc) as tc, ExitStack() as ctx:
        const = ctx.enter_context(tc.tile_pool(name="const", bufs=1))
        sbA = ctx.enter_context(tc.tile_pool(name="sbA", bufs=1))
        peA_p = ctx.enter_context(tc.tile_pool(name="peA", bufs=5))
        peD_p = ctx.enter_context(tc.tile_pool(name="peD", bufs=5))
        psml = ctx.enter_context(tc.tile_pool(name="psml", bufs=3))
        dram = ctx.enter_context(tc.tile_pool(name="dram", bufs=1,
                                              space="DRAM"))
        psO = ctx.enter_context(tc.tile_pool(name="psO", bufs=2,
                                             space="PSUM"))

        # ---- constants / persistent SBUF ----
        wqkvT_sb = const.tile([128, 2, 320], BF16)
        nc.sync.dma_start(wqkvT_sb[:],
                          wqkvT_d.rearrange("(c p) m -> p c m", p=128))
        woT_sb = const.tile([128, 4, DIM], BF16)
        nc.sync.dma_start(woT_sb[:],
                          woT_d.rearrange("(c p) m -> p c m", p=128))
        b_sb = const.tile([128, 2], F32)
        nc.sync.dma_start(b_sb[:], b_d.rearrange("(m p) -> p m", p=128))

        ones_sb = const.tile([128, HD], BF16)
        nc.vector.memset(ones_sb[:], 1.0)
        expb_sb = const.tile([128, 1], F32)
        nc.vector.memset(expb_sb[:], EXP_BIAS)
        x_sb = sbA.tile([128, 2, N], BF16)
        qk_sb = sbA.tile([128, N], BF16)   # partitions 0:64 = q, 64:128 = k
        kq_sb = sbA.tile([128, N], BF16)   # partitions 0:64 = k, 64:128 = q
        # v^T augmented with a ones column: [j, 0:64] = v^T, [j, 64] = 1
        vaug_sb = sbA.tile([128, NJ, 66], BF16)
        nc.vector.memset(vaug_sb[:, :, 64:65], 1.0)
        # fp8 copy of v^T for the DoubleRow out' pairs (tiles 4u, 4u+1):
        # the unit's two K-planes side by side; 80-col plane stride keeps
        # the DR weight-AP step a multiple of 16
        vaug8_sb = sbA.tile([128, NJ // 4, 2, 80], FP8)
        nc.vector.memset(vaug8_sb[:, :, :, 64:65], 1.0)

        a2a_in = dram.tile([NB, HD, BLK], BF16)
        a2a_out = dram.tile([NB, HD, BLK], BF16)
        warm_in = dram.tile([128, 4], F32)
        warm_out = dram.tile([128, 4], F32)
        # tiny warm-up collective: absorbs CC init cost under the preamble
        # (an extra warm-up AllToAll measurably SLOWS the real one - don't)
        nc.gpsimd.collective_compute(
            "AllReduce", mybir.AluOpType.add,
            replica_groups=[list(range(N_CORES))],
            ins=[warm_in.opt()], outs=[warm_out.opt()])

        pending = []  # [countdown, fn] emitted in order once countdown <= 0

        def schedule(fn, delay):
            pending.append([delay, fn])

        def tick():
            for it in pending:
                it[0] -= 1
            for it in [it for it in pending if it[0] <= 0]:
                pending.remove(it)
                it[1]()

        def drain():
            while pending:
                pending.pop(0)[1]()

        def emit_stage_a(b, pool):
            bs = slice(b * BLK, (b + 1) * BLK)
            # x loads run one block ahead (block b+1 issued here; block 0
            # hoisted before the loop) so the projections never wait on DMA
            if b + 1 < NB:
                nbs = slice((b + 1) * BLK, (b + 2) * BLK)
                for c in range(2):
                    nc.sync.dma_start(x_sb[:, c, nbs],
                                      x_d[c * 128:(c + 1) * 128, nbs])
            # qk in slot 0 of a shared pair tile; the 4 v accumulators fit
            # in the first half of slot 1 (1 KB of the 2 KB bank)
            ps = pool.tile([128, 2, BLK], F32, tag="psg", name=f"psa_{b}")
            ps_qk = ps[:, 0, :]
            for c in range(2):
                nc.tensor.matmul(ps_qk, wqkvT_sb[:, c, 0:128],
                                 x_sb[:, c, bs],
                                 start=(c == 0), stop=(c == 1))
            nc.vector.tensor_copy(qk_sb[:, bs], ps_qk)
            # dual layout by partition-swap DMA instead of 2 extra matmuls
            # (split across sync/scalar queues; gpsimd would stall them
            # behind the warm-up collective's rendezvous)
            nc.sync.dma_start(kq_sb[0:64, bs], qk_sb[64:128, bs])
            nc.scalar.dma_start(kq_sb[64:128, bs], qk_sb[0:64, bs])
            ps_v = ps[:, 1, 0:4 * HD].rearrange("p (t d) -> p t d", t=4)
            for t in range(4):
                nt = b * 4 + t
                for c in range(2):
                    nc.tensor.matmul(
                        ps_v[:, t, :],
                        x_sb[:, c, nt * 128:(nt + 1) * 128],
                        wqkvT_sb[:, c, 256:320],
                        start=(c == 0), stop=(c == 1))
            nc.vector.tensor_copy(vaug_sb[:, 4 * b + 2:4 * b + 4, 0:64],
                                  ps_v[:, 2:4, :])
            nc.scalar.copy(vaug8_sb[:, b, :, 0:64], ps_v[:, 0:2, :])

        def emit_sim(i, j, psg, t):
            # sim matmul, row-packed by parity: even j on array rows 0:64,
            # odd j on rows 64:128 -> each (even, odd) pair streams
            # concurrently through the PE.
            isl = slice(i * BLK, (i + 1) * BLK)
            if j % 2 == 0:
                nc.tensor.matmul(psg[:, t, :],
                                 kq_sb[0:64, j * 128:(j + 1) * 128],
                                 qk_sb[0:64, isl],
                                 start=True, stop=True,
                                 tile_position=(0, 0))
            else:
                nc.tensor.matmul(psg[:, t, :],
                                 qk_sb[64:128, j * 128:(j + 1) * 128],
                                 kq_sb[64:128, isl],
                                 start=True, stop=True,
                                 tile_position=(64, 0))

        def emit_exp_outs(i, js, nA, psg, ps_out, delay):
            # exp on ScalarE for the first nA tiles (activation scale
            # un-does the host pre-scale), Schraudolph fast-exp on VectorE
            # for the rest, then the out' accumulation matmuls (delayed).
            gsz = len(js)
            nD = gsz - nA
            pea = ped = None
            if nD > 0:
                ped = peD_p.tile([128, 3, BLK], I16, tag="ped",
                                 name=f"ped_{i}_{js[0]}")
                nc.vector.tensor_scalar_add(ped[:, 0:nD, :],
                                            psg[:, nA:gsz, :], SCH_BIAS)
            if nA > 0:
                pea = peA_p.tile([128, 3, BLK], FP8, tag="pea",
                                 name=f"pea_{i}_{js[0]}")
                nc.scalar.activation(pea[:, 0:nA, :], psg[:, 0:nA, :], EXP,
                                     bias=expb_sb[:, 0:1], scale=ACT_SCALE)

            def mk_outp():
                if nA == 2:
                    # one fp8 DoubleRow matmul covers both ACT tiles: the
                    # two K-planes (j, j+1) stream interleaved at 2 fp8
                    # MACs/cell/cycle (~1.44x over two bf16 matmuls)
                    u2 = js[0] // 4
                    nc.tensor.matmul(ps_out[0:65, :],
                                     vaug8_sb[:, u2, :, 0:65],
                                     pea[:, 0:2, :],
                                     start=(js[0] == 0), stop=False,
                                     perf_mode=mybir.MatmulPerfMode.DoubleRow)
                else:
                    for t2, j2 in enumerate(js):
                        rhs = ped[:, t2 - nA, :].bitcast(BF16)
                        nc.tensor.matmul(ps_out[0:65, :],
                                         vaug_sb[:, j2, 0:65],
                                         rhs,
                                         start=False, stop=(j2 == NJ - 1))
            schedule(mk_outp, delay)

        def emit_unit(i, u, ps_out, pool):
            # one unit of 4 j-tiles: two (even, odd) sim pairs into two
            # 2-slot psg tiles (a pair shares a tile, so the scheduler
            # cannot split it), exp for pair A on ScalarE and pair B on
            # VectorE, and the four out' matmuls batched to halve
            # LDWEIGHTS row-conflict stalls at the sim/out boundaries.
            j0 = 4 * u
            psgA = pool.tile([128, 2, BLK], F32, tag="psg",
                             name=f"psgA_{i}_{u}")
            emit_sim(i, j0, psgA, 0)
            emit_sim(i, j0 + 1, psgA, 1)
            psgB = pool.tile([128, 2, BLK], F32, tag="psg",
                             name=f"psgB_{i}_{u}")
            emit_sim(i, j0 + 2, psgB, 0)
            emit_sim(i, j0 + 3, psgB, 1)
            emit_exp_outs(i, [j0, j0 + 1], 2, psgA, ps_out, 2)
            emit_exp_outs(i, [j0 + 2, j0 + 3], 0, psgB, ps_out, 2)
            tick()

        def emit_iblock(i, ps_out, pool):
            for u in range(NJ // 4):
                emit_unit(i, u, ps_out, pool)

        def emit_norm(i, ps_out):
            # row sums live on partition 64 of ps_out; read PSUM directly
            # (psO is double-buffered so the bank frees at leisure).
            s_sb = psml.tile([128, BLK], F32, tag="ssb", name=f"ssb_{i}")

            if i == NB - 1:
                # tail block: the a2a is waiting on this chain, so avoid
                # DRAM bounces entirely.  Spread the sums row over 32
                # lanes with the DVE 32x32 transpose, reciprocal there,
                # transpose back, then replicate with a K=1 bf16 matmul.
                st_sb = psml.tile([128, BLK], F32, tag="stt", name="st_t")
                rt_sb = psml.tile([128, BLK], F32, tag="rtt", name="rt_t")
                rb_sb = psml.tile([128, BLK], F32, tag="rbb", name="rb_t")
                rh_sb = psml.tile([128, BLK], BF16, tag="rhh", name="rh_t")
                oall = psml.tile([HD, BLK], F32, tag="rrep", name="oall_t")

                def mk_tail():
                    nc.vector.tensor_copy(s_sb[64:65, :], ps_out[64:65, :])
                    nc.scalar.copy(oall[:], ps_out[0:HD, :])
                    nc.vector.transpose(st_sb[64:96, :], s_sb[64:96, :])
                    stv = st_sb[64:96, :].rearrange("p (a b) -> p a b", b=32)
                    rtv = rt_sb[64:96, :].rearrange("p (a b) -> p a b", b=32)
                    nc.vector.reciprocal(rtv[:, :, 0:1], stv[:, :, 0:1])
                    nc.vector.transpose(rb_sb[64:96, :], rt_sb[64:96, :])
                    nc.vector.tensor_copy(rh_sb[64:65, :], rb_sb[64:65, :])
                    ps_r = psO.tile([128, BLK], F32, tag="psout",
                                    name="psr_tail")
                    nc.tensor.matmul(ps_r[0:HD, :], ones_sb[64:65, 0:HD],
                                     rh_sb[64:65, :], start=True, stop=True)
                    outn = psml.tile([HD, BLK], BF16, tag="outn",
                                     name="outn_t")
                    nc.vector.tensor_mul(outn[:], oall[:], ps_r[0:HD, :])
                    nc.sync.dma_start(a2a_in[i], outn[:])
                schedule(mk_tail, 2)
                return

            s4_sb = psml.tile([128, 4], F32, tag="s4", name=f"s4_{i}")
            r4_sb = psml.tile([128, 4], F32, tag="r4", name=f"r4_{i}")
            rrow = dram.tile([BLK], F32, tag="rrow", bufs=2,
                             name=f"rrow_{i}")
            rrec = dram.tile([BLK], F32, tag="rrec", bufs=2,
                             name=f"rrec_{i}")

            def mk_norm_a():
                # sums row -> DRAM -> respread over 128 lanes so the
                # reciprocal runs wide instead of on one partition
                nc.vector.tensor_copy(s_sb[64:65, :], ps_out[64:65, :])
                nc.sync.dma_start(rrow[:], s_sb[64:65, :])
            schedule(mk_norm_a, 2)

            def mk_norm_a2():
                nc.sync.dma_start(s4_sb[:],
                                  rrow.rearrange("(p c) -> p c", p=128))
                nc.vector.reciprocal(r4_sb[:], s4_sb[:])
                nc.sync.dma_start(rrec[:], r4_sb[:])
            schedule(mk_norm_a2, 4)

            def mk_norm_b():
                outn = psml.tile([HD, BLK], BF16, tag="outn",
                                 name=f"outn_{i}")
                rrep_sb = psml.tile([HD, BLK], F32, tag="rrep",
                                    name=f"rrep_{i}")
                nc.sync.dma_start(
                    rrep_sb[:],
                    rrec.rearrange("(o n) -> o n", o=1).broadcast_to(
                        (HD, BLK)))
                nc.vector.tensor_mul(outn[:], ps_out[0:HD, :],
                                     rrep_sb[:])
                nc.sync.dma_start(a2a_in[i], outn[:])
            schedule(mk_norm_b, 6)

        # ---- stage A interleaved with i-block 0: unit u of i0 needs
        # only stage-A block u's outputs, so the i0 units slot in one
        # block behind the projections (all in the shared psB3 pool) -----
        with tc.tile_pool(name="psB3", bufs=3, space="PSUM") as psB3:
            # warm the PE clock-gate during the preamble DMA wait so the
            # projections run at 2.4 GHz from block 0
            ps_w = psB3.tile([128, 2, BLK], F32, tag="psg", name="ps_warmup")
            for w in range(12):
                nc.tensor.matmul(ps_w[:, 0, 0:320], wqkvT_sb[:, 0, 0:128],
                                 wqkvT_sb[:, 1, :], start=True, stop=True)
            ps_out0 = psO.tile([128, BLK], F32, tag="psout", name="psout_0")
            for c in range(2):
                nc.sync.dma_start(x_sb[:, c, 0:BLK],
                                  x_d[c * 128:(c + 1) * 128, 0:BLK])
            u0 = 0
            for b in range(NB):
                emit_stage_a(b, psB3)
                if b >= 1:
                    emit_unit(0, u0, ps_out0, psB3)
                    u0 += 1
            while u0 < NJ // 4:
                emit_unit(0, u0, ps_out0, psB3)
                u0 += 1
            emit_norm(0, ps_out0)

            # ---- i-blocks 1..7 ------------------------------------------
            for i in range(1, NB):
                ps_out = psO.tile([128, BLK], F32, tag="psout",
                                  name=f"psout_{i}")
                emit_iblock(i, ps_out, psB3)
                emit_norm(i, ps_out)
            drain()

            # ---- stage C: AllToAll over token blocks + output projection --
            nc.gpsimd.collective_compute(
                "AllToAll", mybir.AluOpType.bypass,
                replica_groups=[list(range(N_CORES))],
                ins=[a2a_in.opt()], outs=[a2a_out.opt()])

            rhs_sb = sbA.tile([128, 4, BLK], BF16)
            a2a_r = a2a_out.rearrange("(c a) d t -> (a d) c t", c=4, a=2)
            ps_yt = psB3.tile([128, 2, BLK], F32, tag="psg", name="psy")
            # fan the 4 chunk loads over both hardware DGE queues so they
            # run concurrently instead of serializing on the gpsimd queue
            load_qs = [nc.sync, nc.scalar, nc.sync, nc.scalar]
            for c in range(4):
                load_qs[c].dma_start(rhs_sb[:, c, :], a2a_r[:, c, :])
            for c in range(4):
                for m in range(2):
                    nc.tensor.matmul(ps_yt[:, m, :],
                                     woT_sb[:, c, m * 128:(m + 1) * 128],
                                     rhs_sb[:, c, :],
                                     start=(c == 0), stop=(c == 3))
            store_qs = [nc.sync, nc.scalar]
            for m in range(2):
                y_sb = psml.tile([128, BLK], F32, tag="ysb", name=f"ysb_{m}")
                nc.vector.tensor_scalar_add(y_sb[:], ps_yt[:, m, :],
                                            b_sb[:, m:m + 1])
                store_qs[m].dma_start(y_d[m * 128:(m + 1) * 128, :], y_sb[:])

    nc.compile()
    return nc


def _make_in_maps(x, w_qkv, w_out, b_out):
    x2 = np.ascontiguousarray(
        np.asarray(x, np.float32).reshape(DIM, N)).astype(ml_dtypes.bfloat16)
    w_qkv = np.asarray(w_qkv, np.float32)
    scale = (HD ** -0.5) * SCH_SCALE
    woT = np.ascontiguousarray(np.asarray(w_out, np.float32).T).astype(
        ml_dtypes.bfloat16)
    b = np.ascontiguousarray(np.asarray(b_out, np.float32).reshape(DIM))
    in_maps = []
    for h in range(N_CORES):
        wq = w_qkv[h * HD:(h + 1) * HD] * scale
        wk = w_qkv[HID + h * HD:HID + (h + 1) * HD]
        wv = w_qkv[2 * HID + h * HD:2 * HID + (h + 1) * HD]
        wqkvT = np.ascontiguousarray(
            np.concatenate([wq.T, wk.T, wk.T, wq.T, wv.T], axis=1),
            np.float32).astype(ml_dtypes.bfloat16)
        in_maps.append({"x": x2, "wqkvT": wqkvT, "woT": woT, "bout": b})
    return in_maps


def _assemble(results):
    y = np.concatenate([results[h]["y"] for h in range(N_CORES)], axis=1)
    return np.ascontiguousarray(y.reshape(1, DIM, 64, 64).astype(np.float32))


def kernel(x, w_qkv, w_out, b_out):
    nc = build_program()
    in_maps = _make_in_maps(x, w_qkv, w_out, b_out)
    res = run_bass_kernel_spmd(nc, in_maps, list(range(N_CORES)))
    return _assemble(res.results)


def run_traced(x, w_qkv, w_out, b_out, trace_cores=None):
    """Test-harness entry: also returns BassKernelResults with exec_time_ns."""
    nc = build_program()
    in_maps = _make_in_maps(x, w_qkv, w_out, b_out)
    res = run_bass_kernel_spmd(nc, in_maps, list(range(N_CORES)), trace=True,
                               trace_cores=trace_cores)
    return _assemble(res.results), res

